# revision 11
# baseline (speedup 1.0000x reference)
"""SC-LSTM decoder (2-layer, teacher-forced) Trainium2 Bass kernel.

Strategy (8 NeuronCores):
  - Tensor-parallel over the hidden dimension: core j owns H-rows
    [128j, 128j+128) of each layer (and V-cols [256j, 256j+256) of the
    output projection). Full batch B=128 stays on every core, which
    exactly fills the PE stationary dimension.
  - Phase A (parallel): precompute all x-dependent GEMM contributions
    (gx0 = x@w2h_W0, gx1 = x@w2h_W1[:E], rx = x@w2hr) for all T steps,
    column-sharded across cores.  X is pre-transposed on the host so the
    contraction dim (E) lands on SBUF partitions.
  - Phase C (sequential over T): per step, each core computes its H-slice
    of the gates from SBUF-resident weight slices, the cell update, and
    its new hidden slice; two small AllGathers (64KB/rank) rebuild the
    full transposed hidden state h^T for the next step's contractions.
    The output projection slice runs inside the loop off the gathered
    h^T tiles.
"""

import sys

sys.path.insert(0, "/opt/trn_rl_repo")

import numpy as np

import concourse.bass as bass
import concourse.mybir as mybir
import concourse.tile as tile
from concourse import bacc
from concourse.bass_utils import run_bass_kernel_spmd
from concourse.masks import make_identity

B, T, E, H, D, V, L = 128, 100, 2048, 1024, 256, 2048, 2
NC = 8
P = 128
HS = H // NC      # 128 h-rows per core per layer
GS = 4 * HS       # 512 packed gate cols per core
VS = V // NC      # 256 output cols per core
KE = E // P       # 16 k-tiles over E
KH = H // P       # 8 k-tiles over H
F32 = mybir.dt.float32

_cache = {}


def _build(t_steps: int):
    nc = bacc.Bacc("TRN2", target_bir_lowering=False, debug=False, num_devices=NC)

    # ---------------- I/O declarations (per-core values supplied via in_maps)
    xT = nc.dram_tensor("xT", [E, t_steps * B], F32, kind="ExternalInput")
    h0T_i = nc.dram_tensor("h0T_i", [H, B], F32, kind="ExternalInput")
    c_i = nc.dram_tensor("c_i", [B, HS], F32, kind="ExternalInput")
    d_i = nc.dram_tensor("d_i", [B, D], F32, kind="ExternalInput")
    # precompute weights (col-sliced per core)
    Wx0 = nc.dram_tensor("Wx0", [E, GS], F32, kind="ExternalInput")
    Wx1x = nc.dram_tensor("Wx1x", [E, GS], F32, kind="ExternalInput")
    Wrx = nc.dram_tensor("Wrx", [E, 2 * D], F32, kind="ExternalInput")
    # recurrence weights
    Wh0 = nc.dram_tensor("Wh0", [H, GS], F32, kind="ExternalInput")
    Wh1 = nc.dram_tensor("Wh1", [H, GS], F32, kind="ExternalInput")
    Wx1h = nc.dram_tensor("Wx1h", [H, GS], F32, kind="ExternalInput")
    Wrc = nc.dram_tensor("Wrc", [2 * H, D], F32, kind="ExternalInput")
    Wr1h = nc.dram_tensor("Wr1h", [H, D], F32, kind="ExternalInput")
    Wdc0 = nc.dram_tensor("Wdc0", [D, HS], F32, kind="ExternalInput")
    Wdc1 = nc.dram_tensor("Wdc1", [D, HS], F32, kind="ExternalInput")
    Wout = nc.dram_tensor("Wout", [2 * H, VS], F32, kind="ExternalInput")

    out_o = nc.dram_tensor("out", [t_steps, B, VS], F32, kind="ExternalOutput")

    # DRAM scratch for the precomputed x-contributions
    gx0d = nc.dram_tensor("gx0d", [t_steps, B, GS], F32)
    gx1d = nc.dram_tensor("gx1d", [t_steps, B, GS], F32)
    rxd = nc.dram_tensor("rxd", [t_steps, B, 2 * D], F32)

    rg = [list(range(NC))]

    with tile.TileContext(nc) as tc:
        with tc.tile_pool(name="const", bufs=1) as constp:
            ident = constp.tile([P, P], F32)
            make_identity(nc, ident[:])

            # ---------------- Phase A: precompute x-contributions
            with (
                tc.tile_pool(name="wx", bufs=1) as wxp,
                tc.tile_pool(name="xa", bufs=3) as xap,
                tc.tile_pool(name="ga", bufs=2) as gap,
                tc.tile_pool(name="psa", bufs=2, space="PSUM") as psa,
            ):
                wx0 = wxp.tile([P, KE, GS], F32)
                wx1 = wxp.tile([P, KE, GS], F32)
                wrx = wxp.tile([P, KE, 2 * D], F32)
                nc.sync.dma_start(wx0[:], Wx0.rearrange("(k p) n -> p k n", p=P))
                nc.sync.dma_start(wx1[:], Wx1x.rearrange("(k p) n -> p k n", p=P))
                nc.sync.dma_start(wrx[:], Wrx.rearrange("(k p) n -> p k n", p=P))

                for t in range(t_steps):
                    xt = xap.tile([P, KE, B], F32, tag="xt")
                    nc.sync.dma_start(
                        xt[:],
                        xT[:, t * B : (t + 1) * B].rearrange(
                            "(k p) n -> p k n", p=P
                        ),
                    )
                    g0p = psa.tile([B, GS], F32, tag="g0p", bufs=2)
                    g1p = psa.tile([B, GS], F32, tag="g1p", bufs=2)
                    rxp = psa.tile([B, 2 * D], F32, tag="rxp", bufs=2)
                    for k in range(KE):
                        st, sp = (k == 0), (k == KE - 1)
                        nc.tensor.matmul(g0p[:], xt[:, k, :], wx0[:, k, :], start=st, stop=sp)
                        nc.tensor.matmul(g1p[:], xt[:, k, :], wx1[:, k, :], start=st, stop=sp)
                        nc.tensor.matmul(rxp[:], xt[:, k, :], wrx[:, k, :], start=st, stop=sp)
                    g0s = gap.tile([B, GS], F32, tag="g0s")
                    g1s = gap.tile([B, GS], F32, tag="g1s")
                    rxs = gap.tile([B, 2 * D], F32, tag="rxs")
                    nc.vector.tensor_copy(g0s[:], g0p[:])
                    nc.vector.tensor_copy(g1s[:], g1p[:])
                    nc.vector.tensor_copy(rxs[:], rxp[:])
                    nc.sync.dma_start(gx0d[t], g0s[:])
                    nc.sync.dma_start(gx1d[t], g1s[:])
                    nc.sync.dma_start(rxd[t], rxs[:])

            # ---------------- Phase B/C: recurrence
            with (
                tc.tile_pool(name="wr", bufs=1) as wrp,
                tc.tile_pool(name="st", bufs=2) as stp,
                tc.tile_pool(name="gx", bufs=3) as gxp,
                tc.tile_pool(name="wk", bufs=2) as wkp,
                tc.tile_pool(name="psg", bufs=2, space="PSUM") as psg,
                tc.tile_pool(name="psr", bufs=2, space="PSUM") as psr,
                tc.tile_pool(name="pst", bufs=2, space="PSUM") as pst,
                tc.tile_pool(name="dma_b", bufs=4, space="DRAM") as dramp,
            ):
                wh0 = wrp.tile([P, KH, GS], F32)
                wh1 = wrp.tile([P, KH, GS], F32)
                wx1h = wrp.tile([P, KH, GS], F32)
                wrc = wrp.tile([P, 2 * KH, D], F32)
                wr1h = wrp.tile([P, KH, D], F32)
                wdc0 = wrp.tile([P, D // P, HS], F32)
                wdc1 = wrp.tile([P, D // P, HS], F32)
                wout = wrp.tile([P, 2 * KH, VS], F32)
                nc.sync.dma_start(wh0[:], Wh0.rearrange("(k p) n -> p k n", p=P))
                nc.sync.dma_start(wh1[:], Wh1.rearrange("(k p) n -> p k n", p=P))
                nc.sync.dma_start(wx1h[:], Wx1h.rearrange("(k p) n -> p k n", p=P))
                nc.sync.dma_start(wrc[:], Wrc.rearrange("(k p) n -> p k n", p=P))
                nc.sync.dma_start(wr1h[:], Wr1h.rearrange("(k p) n -> p k n", p=P))
                nc.sync.dma_start(wdc0[:], Wdc0.rearrange("(k p) n -> p k n", p=P))
                nc.sync.dma_start(wdc1[:], Wdc1.rearrange("(k p) n -> p k n", p=P))
                nc.sync.dma_start(wout[:], Wout.rearrange("(k p) n -> p k n", p=P))

                h0T = stp.tile([P, KH, B], F32, tag="h0T")
                h1T = stp.tile([P, KH, B], F32, tag="h1T")
                nc.sync.dma_start(h0T[:], h0T_i.rearrange("(k p) n -> p k n", p=P))
                nc.sync.dma_start(h1T[:], h0T_i.rearrange("(k p) n -> p k n", p=P))
                c0 = stp.tile([B, HS], F32, tag="c0")
                c1 = stp.tile([B, HS], F32, tag="c1")
                nc.sync.dma_start(c0[:], c_i[:])
                nc.sync.dma_start(c1[:], c_i[:])
                d0 = stp.tile([B, D], F32, tag="d0")
                d1 = stp.tile([B, D], F32, tag="d1")
                nc.sync.dma_start(d0[:], d_i[:])
                nc.sync.dma_start(d1[:], d_i[:])

                Sig = mybir.ActivationFunctionType.Sigmoid
                Tanh = mybir.ActivationFunctionType.Tanh
                mul = mybir.AluOpType.mult
                add = mybir.AluOpType.add

                def layer(t, li, gx, rx_sl, rcp, r1p, hT_stat, wg, c_cur, d_cur,
                          wdc):
                    """One layer's gate + cell update. Returns (nh, c_new, d_new).

                    gx: SBUF (B, GS) x-contribution; rx_sl: SBUF (B, D) slice;
                    rcp: PSUM (B, D) shared r-gate contribution;
                    r1p: PSUM (B, D) extra r contribution or None;
                    hT_stat: list of (ap, k) stationary tiles for the gate GEMM;
                    wg: list of matching moving weight APs.
                    """
                    gp = psg.tile([B, GS], F32, tag="gp", bufs=1)
                    n = len(hT_stat)
                    for i, (hs_ap, w_ap) in enumerate(zip(hT_stat, wg)):
                        nc.tensor.matmul(gp[:], hs_ap, w_ap, start=(i == 0), stop=(i == n - 1))
                    gsum = wkp.tile([B, GS], F32, tag=f"gsum{li}")
                    nc.vector.tensor_tensor(gsum[:], gp[:], gx[:], add)
                    # activations: packed cols [i|f|o|c]
                    sig = wkp.tile([B, 3 * HS], F32, tag=f"sig{li}")
                    nc.scalar.activation(sig[:], gsum[:, : 3 * HS], Sig)
                    tgc = wkp.tile([B, HS], F32, tag=f"tgc{li}")
                    nc.scalar.activation(tgc[:], gsum[:, 3 * HS :], Tanh)

                    # r gate
                    rsum = wkp.tile([B, D], F32, tag=f"rsum{li}")
                    # DVE has a single PSUM read port: at most one PSUM operand per op
                    nc.vector.tensor_tensor(rsum[:], rcp[:], rx_sl, add)
                    if r1p is not None:
                        nc.vector.tensor_tensor(rsum[:], rsum[:], r1p[:], add)
                    nc.scalar.activation(rsum[:], rsum[:], Sig)
                    d_new = stp.tile([B, D], F32, tag=f"d{li}")
                    nc.vector.tensor_tensor(d_new[:], rsum[:], d_cur[:], mul)
                    # transpose dt -> (D, B) for the dc GEMM
                    dtT_p = pst.tile([P, D // P, B], F32, tag="dtTp", bufs=1)
                    for k in range(D // P):
                        nc.tensor.transpose(dtT_p[:, k, :], d_new[:, k * P : (k + 1) * P], ident[:])
                    dtT = wkp.tile([P, D // P, B], F32, tag=f"dtT{li}")
                    nc.vector.tensor_copy(dtT[:], dtT_p[:])
                    dcp = psr.tile([B, HS], F32, tag="dcp", bufs=1)
                    for k in range(D // P):
                        nc.tensor.matmul(dcp[:], dtT[:, k, :], wdc[:, k, :], start=(k == 0), stop=(k == D // P - 1))
                    tdc = wkp.tile([B, HS], F32, tag=f"tdc{li}")
                    nc.scalar.activation(tdc[:], dcp[:], Tanh)

                    # cell = gf*c + gi*tanh(gc) + tanh(dc)
                    c_new = stp.tile([B, HS], F32, tag=f"c{li}")
                    m1 = wkp.tile([B, HS], F32, tag=f"m1{li}")
                    nc.vector.tensor_tensor(m1[:], sig[:, :HS], tgc[:], mul)
                    nc.vector.tensor_tensor(c_new[:], sig[:, HS : 2 * HS], c_cur[:], mul)
                    nc.vector.tensor_tensor(c_new[:], c_new[:], m1[:], add)
                    nc.vector.tensor_tensor(c_new[:], c_new[:], tdc[:], add)
                    # nh = go * tanh(c_new)
                    nh = wkp.tile([B, HS], F32, tag=f"nh{li}")
                    nc.scalar.activation(nh[:], c_new[:], Tanh)
                    nc.vector.tensor_tensor(nh[:], sig[:, 2 * HS : 3 * HS], nh[:], mul)
                    return nh, c_new, d_new

                def gather_hT(nh, li):
                    """Transpose own nh slice and AllGather into full h^T tiles."""
                    nhT_p = pst.tile([P, B], F32, tag="nhTp", bufs=2)
                    nc.tensor.transpose(nhT_p[:], nh[:], ident[:])
                    nhT = wkp.tile([P, B], F32, tag=f"nhT{li}")
                    nc.vector.tensor_copy(nhT[:], nhT_p[:])
                    agi = dramp.tile([P, B], F32, tag=f"agi{li}")
                    ago = dramp.tile([H, B], F32, tag=f"ago{li}", addr_space="Shared")
                    nc.sync.dma_start(agi[:], nhT[:])
                    nc.gpsimd.collective_compute(
                        "AllGather", mybir.AluOpType.bypass, replica_groups=rg,
                        ins=[agi[:]], outs=[ago[:]],
                    )
                    hT_new = stp.tile([P, KH, B], F32, tag=f"h{li}T")
                    nc.sync.dma_start(hT_new[:], ago.rearrange("(k p) n -> p k n", p=P))
                    return hT_new

                for t in range(t_steps):
                    gx0 = gxp.tile([B, GS], F32, tag="gx0")
                    gx1 = gxp.tile([B, GS], F32, tag="gx1")
                    rx = gxp.tile([B, 2 * D], F32, tag="rx")
                    nc.sync.dma_start(gx0[:], gx0d[t])
                    nc.sync.dma_start(gx1[:], gx1d[t])
                    nc.sync.dma_start(rx[:], rxd[t])

                    # r_common = [h0;h1]^T-contraction with alpha-folded weights
                    rcp = psr.tile([B, D], F32, tag="rcp", bufs=1)
                    for k in range(2 * KH):
                        src = h0T[:, k, :] if k < KH else h1T[:, k - KH, :]
                        nc.tensor.matmul(rcp[:], src, wrc[:, k, :], start=(k == 0), stop=(k == 2 * KH - 1))

                    # ---- layer 0
                    nh0, c0, d0 = layer(
                        t, 0, gx0, rx[:, :D], rcp, None,
                        [h0T[:, k, :] for k in range(KH)],
                        [wh0[:, k, :] for k in range(KH)],
                        c0, d0, wdc0,
                    )
                    h0T_new = gather_hT(nh0, 0)

                    # r1 extra: nh0 (full, gathered) @ w2hr_W1[E:]
                    r1p = psr.tile([B, D], F32, tag="r1p", bufs=1)
                    for k in range(KH):
                        nc.tensor.matmul(r1p[:], h0T_new[:, k, :], wr1h[:, k, :], start=(k == 0), stop=(k == KH - 1))

                    # ---- layer 1: g1 = gx1 + h1-part + nh0-part
                    nh1, c1, d1 = layer(
                        t, 1, gx1, rx[:, D:], rcp, r1p,
                        [h1T[:, k, :] for k in range(KH)]
                        + [h0T_new[:, k, :] for k in range(KH)],
                        [wh1[:, k, :] for k in range(KH)]
                        + [wx1h[:, k, :] for k in range(KH)],
                        c1, d1, wdc1,
                    )
                    h1T_new = gather_hT(nh1, 1)

                    # ---- output projection slice off the gathered h^T
                    op = psg.tile([B, VS], F32, tag="op", bufs=1)
                    for k in range(2 * KH):
                        src = h0T_new[:, k, :] if k < KH else h1T_new[:, k - KH, :]
                        nc.tensor.matmul(op[:], src, wout[:, k, :], start=(k == 0), stop=(k == 2 * KH - 1))
                    osb = wkp.tile([B, VS], F32, tag="osb")
                    nc.vector.tensor_copy(osb[:], op[:])
                    nc.sync.dma_start(out_o[t], osb[:])

                    h0T, h1T = h0T_new, h1T_new

    nc.compile()
    return nc


def _prep_inputs(input_seq, h0, dt0, w2h_W0, w2h_b0, w2h_W1, w2h_b1,
                 w2hr_W0, w2hr_b0, w2hr_W1, w2hr_b1,
                 h2h_W0, h2h_b0, h2h_W1, h2h_b1,
                 h2hr_W0, h2hr_b0, h2hr_W1, h2hr_b1,
                 dc_W0, dc_W1, out_W, out_b, t_steps):
    f = np.float32
    for name, b in [("w2h_b0", w2h_b0), ("w2h_b1", w2h_b1), ("w2hr_b0", w2hr_b0),
                    ("w2hr_b1", w2hr_b1), ("h2h_b0", h2h_b0), ("h2h_b1", h2h_b1),
                    ("h2hr_b0", h2hr_b0), ("h2hr_b1", h2hr_b1), ("out_b", out_b)]:
        assert not np.any(np.asarray(b)), f"nonzero bias {name} unsupported"

    # time-step inputs: SOS one-hot at t=0, then input_seq[:, t-1]
    xs = np.empty((t_steps, B, E), f)
    xs[0] = 0.0
    xs[0, :, 0] = 1.0
    xs[1:] = np.asarray(input_seq, f).transpose(1, 0, 2)[: t_steps - 1]
    xT = np.ascontiguousarray(xs.reshape(t_steps * B, E).T)

    h0 = np.asarray(h0, f)
    h0T = np.ascontiguousarray(h0.T)
    dt0 = np.asarray(dt0, f)

    alpha = 1.0 / L
    wrc_full = np.concatenate([np.asarray(h2hr_W0, f), np.asarray(h2hr_W1, f)], 0) * alpha
    wrx_full = np.concatenate([np.asarray(w2hr_W0, f), np.asarray(w2hr_W1, f)[:E]], 1)

    in_maps = []
    for j in range(NC):
        gc = np.r_[tuple(np.arange(g * H + j * HS, g * H + (j + 1) * HS) for g in range(4))]
        vs = slice(j * VS, (j + 1) * VS)
        in_maps.append({
            "xT": xT,
            "h0T_i": h0T,
            "c_i": np.ascontiguousarray(h0[:, j * HS : (j + 1) * HS]),
            "d_i": dt0,
            "Wx0": np.ascontiguousarray(np.asarray(w2h_W0, f)[:, gc]),
            "Wx1x": np.ascontiguousarray(np.asarray(w2h_W1, f)[:E, gc]),
            "Wrx": wrx_full,
            "Wh0": np.ascontiguousarray(np.asarray(h2h_W0, f)[:, gc]),
            "Wh1": np.ascontiguousarray(np.asarray(h2h_W1, f)[:, gc]),
            "Wx1h": np.ascontiguousarray(np.asarray(w2h_W1, f)[E:, gc]),
            "Wrc": wrc_full,
            "Wr1h": np.ascontiguousarray(np.asarray(w2hr_W1, f)[E:]),
            "Wdc0": np.ascontiguousarray(np.asarray(dc_W0, f)[:, j * HS : (j + 1) * HS]),
            "Wdc1": np.ascontiguousarray(np.asarray(dc_W1, f)[:, j * HS : (j + 1) * HS]),
            "Wout": np.ascontiguousarray(np.asarray(out_W, f)[:, vs]),
        })
    return in_maps


def _run(t_steps, trace, **inputs):
    if trace:
        import prof_shim

        prof_shim.install()
    key = t_steps
    if key not in _cache:
        _cache[key] = _build(t_steps)
    nc = _cache[key]
    in_maps = _prep_inputs(**inputs, t_steps=t_steps)
    res = run_bass_kernel_spmd(nc, in_maps, list(range(NC)), trace=trace)
    parts = [res.results[j]["out"] for j in range(NC)]  # each (T, B, VS)
    full = np.concatenate(parts, axis=2)                # (T, B, V)
    return np.ascontiguousarray(full.transpose(1, 0, 2)), res


def kernel(**inputs) -> np.ndarray:
    out, _ = _run(T, False, **inputs)
    return out


def kernel_traced(t_steps=T, **inputs):
    out, res = _run(t_steps, True, **inputs)
    return out, res
